# revision 33
# baseline (speedup 1.0000x reference)
"""Kitsune (ensemble of tiny autoencoders) Bass kernel for Trainium2, 8 NeuronCores.

Strategy (pure data parallel, batch sharded 8 ways, 65536 rows/core).

The end-to-end wall clock is dominated by the host<->device wire (~65 MB/s
through the PJRT tunnel), so the kernel is built around minimizing bytes
moved per call:
  - x is quantized host-side to uint8 (x8 = round(x*255)) and pre-transposed
    to feature-major [100, R] per core: 50 MB upload instead of 200 MB.
    Dequantization (and the tail min-max normalisation) is folded into one
    per-partition tensor_scalar on device: xn = x8*(a/255) + c.
  - Only t leaves the device, as uint8 [10, R] per core scaled by 255:
    5.25 MB download instead of 42. x_hat is a pure function of t (the tiny
    head MLP), recomputed from the quantized t on XLA-CPU while the wire is
    busy — cheaper than shipping it, and more accurate than an 8-bit x_hat.
  - The batch is processed in two halves through the same compiled module so
    host quantization, device execution, and the t download of half 0 all
    hide under the uploads (the wire is effectively a single ~65 MB/s
    channel, so only non-wire work can be overlapped).
  - The jit wrapper around the bass_exec custom call is built once and
    cached; replicated parameters are uploaded once (content-hashed) and
    kept on device; the inert "donated zero output" operand the bass_exec
    calling convention requires is created on-device once and reused (the
    kernel writes every output element, so its contents are never read).

On-device compute (unchanged math from the reference):
  - fp16 on-chip compute, fp32 PSUM accumulation.
  - All 10 tail autoencoders run as single block-diagonal matmuls
    (enc [100,80], dec [80,100]) on the normalised xn.
  - Per-cluster RMSE reduce = block-diagonal matmul with 0.1-weighted
    cluster-membership stationary; 12 consecutive 512-row blocks accumulate
    into one PSUM tile [120, 512] at partition offsets 10k (slot-shifted
    stationaries), so the PSUM->SBUF copy amortises 12x.
"""
import sys
sys.path.insert(0, '/opt/trn_rl_repo')

import concurrent.futures as _cf
import hashlib
import numpy as np

import jax
import jax.numpy as jnp
from jax.sharding import Mesh, PartitionSpec, NamedSharding
from jax.experimental.shard_map import shard_map

import concourse.bass as bass
import concourse.bacc as bacc
import concourse.tile as tile
import concourse.mybir as mybir
from concourse import bass2jax

dt = mybir.dt
A = mybir.AluOpType
ACTF = mybir.ActivationFunctionType

N_CORES = 8
B = 524288
C, F, H, HC = 10, 10, 8, 8
D = C * F              # 100
EH = C * H             # 80
R = B // N_CORES       # 65536 rows per core
NH = 4                 # host->device pipeline depth (quarters of the batch)
R2 = R // NH           # 16384 rows per core per kernel invocation
BS = 512               # rows per block
GROUPS = [12, 12, 8]   # blocks per group (stacked in PSUM partitions)
EPS = np.float32(1e-16)

PARAM_KEYS = ("tail_enc_w", "tail_enc_b", "tail_dec_w", "tail_dec_b",
              "tail_nmin", "tail_nmax", "head_enc_w", "head_enc_b",
              "head_dec_w", "head_dec_b", "head_nmin", "head_nmax")

_cached = {}


def _build_module():
    nc = bacc.Bacc(None, target_bir_lowering=False, debug=False,
                   num_devices=N_CORES)
    x_d = nc.dram_tensor("x8", [D, R2], dt.uint8, kind="ExternalInput")
    enc_w_d = nc.dram_tensor("enc_w", [D, EH], dt.float16, kind="ExternalInput")
    dec_w_d = nc.dram_tensor("dec_w", [EH, D], dt.float16, kind="ExternalInput")
    red_w_d = nc.dram_tensor("red_w", [D, 120 * 12], dt.float16, kind="ExternalInput")
    vecs_d = nc.dram_tensor("vecs", [128, 8], dt.float32, kind="ExternalInput")
    out_d = nc.dram_tensor("out8", [R2, 10], dt.uint8, kind="ExternalOutput")

    with tile.TileContext(nc) as tc:
        _kernel_body(nc, tc, x_d, enc_w_d, dec_w_d, red_w_d, vecs_d, out_d)
    nc.finalize()
    return nc


def _kernel_body(nc, tc, x_d, enc_w_d, dec_w_d, red_w_d, vecs_d, out_d):
    from contextlib import ExitStack
    with ExitStack() as ctx:
        const = ctx.enter_context(tc.tile_pool(name="const", bufs=1))
        # -- load parameters once --
        enc_w = const.tile([D, EH], dt.float16)
        nc.sync.dma_start(enc_w[:], enc_w_d.ap())
        dec_w = const.tile([EH, D], dt.float16)
        nc.sync.dma_start(dec_w[:], dec_w_d.ap())
        red_w = const.tile([D, 120 * 12], dt.float16)
        nc.sync.dma_start(red_w[:], red_w_d.ap())
        vecs = const.tile([128, 8], dt.float32)
        nc.sync.dma_start(vecs[:], vecs_d.ap())
        # vecs columns: 0=a_vec[100] (has /255 dequant folded), 1=c_vec[100],
        #               2=enc_b[80], 3=dec_b[100],
        #               6=at_vec[120] (*255), 7=ct_vec[120] (*255 + 0.5)
        a_v = vecs[0:D, 0:1]
        c_v = vecs[0:D, 1:2]
        enc_b = vecs[0:EH, 2:3]
        dec_b = vecs[0:D, 3:4]
        at_v = vecs[0:120, 6:7]
        ct_v = vecs[0:120, 7:8]

        Spool = ctx.enter_context(tc.tile_pool(name="Sbuf", bufs=len(GROUPS)))
        S_tiles = []

        # ---------------- phase 1 ----------------
        with tc.tile_pool(name="xt", bufs=3) as xtp, \
             tc.tile_pool(name="act", bufs=2) as actp, \
             tc.tile_pool(name="z1", bufs=2, space="PSUM") as z1p, \
             tc.tile_pool(name="z2", bufs=1, space="PSUM") as z2p, \
             tc.tile_pool(name="Sps", bufs=2, space="PSUM") as Sp:
            blk = 0
            for g, gn in enumerate(GROUPS):
                S_ps = Sp.tile([120, BS], dt.float32, tag="Sps")
                for pp in range(gn // 2):
                    kk = 2 * pp
                    c0 = blk * BS
                    # uint8 feature-major tile straight from DRAM
                    x8 = xtp.tile([D, 2 * BS], dt.uint8, tag="x8")
                    nc.sync.dma_start(x8[:], x_d.ap()[:, c0:c0 + 2 * BS])
                    # dequant + min-max normalise in one op: xn = x8*(a/255)+c
                    xn = actp.tile([D, 2 * BS], dt.float16, tag="xn")
                    nc.vector.tensor_scalar(xn[:], x8[:], a_v, c_v,
                                            A.mult, A.add)
                    # encoder + sigmoid
                    z1 = z1p.tile([EH, 2 * BS], dt.float32, tag="z1")
                    nc.tensor.matmul(z1[:, 0:BS], enc_w[:], xn[:, 0:BS],
                                     start=True, stop=True)
                    nc.tensor.matmul(z1[:, BS:2 * BS], enc_w[:], xn[:, BS:2 * BS],
                                     start=True, stop=True)
                    h = actp.tile([EH, 2 * BS], dt.float16, tag="h")
                    nc.scalar.activation(h[:], z1[:], ACTF.Sigmoid, bias=enc_b)
                    # decoder + sigmoid
                    z2 = z2p.tile([D, 2 * BS], dt.float32, tag="z2")
                    nc.tensor.matmul(z2[:, 0:BS], dec_w[:], h[:, 0:BS],
                                     start=True, stop=True)
                    nc.tensor.matmul(z2[:, BS:2 * BS], dec_w[:], h[:, BS:2 * BS],
                                     start=True, stop=True)
                    rec = actp.tile([D, 2 * BS], dt.float16, tag="rec")
                    nc.scalar.activation(rec[:], z2[:], ACTF.Sigmoid, bias=dec_b)
                    # squared error
                    dd = actp.tile([D, 2 * BS], dt.float16, tag="dd")
                    nc.vector.tensor_tensor(dd[:], rec[:], xn[:], A.subtract)
                    sq = actp.tile([D, 2 * BS], dt.float16, tag="sq")
                    nc.vector.tensor_tensor(sq[:], dd[:], dd[:], A.mult)
                    # per-cluster mean reduce, stacked at partition 10*slot
                    nc.tensor.matmul(S_ps[:], red_w[:, 120 * kk:120 * (kk + 1)],
                                     sq[:, 0:BS], start=(kk == 0), stop=False,
                                     skip_group_check=True)
                    nc.tensor.matmul(S_ps[:], red_w[:, 120 * (kk + 1):120 * (kk + 2)],
                                     sq[:, BS:2 * BS], start=False,
                                     stop=(kk + 1 == gn - 1), skip_group_check=True)
                    blk += 2
                S_sb = Spool.tile([120, BS], dt.float32, tag="Ssb")
                nc.scalar.activation(S_sb[0:10 * gn, :], S_ps[0:10 * gn, :],
                                     ACTF.Copy)
                S_tiles.append(S_sb)

        # ---------------- phase 2: sqrt + quantize t to uint8 ----------------
        with tc.tile_pool(name="tails", bufs=2) as tailp, \
             tc.tile_pool(name="hact", bufs=2) as hactp:
            for g, gn in enumerate(GROUPS):
                P10 = 10 * gn
                tl = tailp.tile([120, BS], dt.float16, tag="tails")
                nc.scalar.activation(tl[0:P10, :], S_tiles[g][0:P10, :], ACTF.Sqrt)
                # t8 = (at*tails + ct)*255 + 0.5, store-cast truncates
                tv8 = hactp.tile([120, BS], dt.uint8, tag="tv8")
                nc.vector.tensor_scalar(tv8[0:P10, :], tl[0:P10, :],
                                        at_v[0:P10, :], ct_v[0:P10, :],
                                        A.mult, A.add)
                # batch-major output: rows are samples, so the host post and
                # the head MLP read it contiguously (no strided transpose);
                # one DMA per slot keeps the access pattern within 3 dims
                row0 = 12 * BS * g
                for k in range(gn):
                    t_ap = out_d.ap()[row0 + k * BS:row0 + (k + 1) * BS, :]
                    nc.sync.dma_start(t_ap.rearrange("j c -> c j"),
                                      tv8[10 * k:10 * (k + 1), :])


def _fold_params(i):
    """Host-side folding of all small parameters. i = inputs dict (np f32)."""
    aw = 1.0 / (i["tail_nmax"].astype(np.float32) - i["tail_nmin"] + EPS)  # [C,F]
    cw = -i["tail_nmin"] * aw
    We = i["tail_enc_w"].astype(np.float32)       # [C,H,F]
    be = i["tail_enc_b"].astype(np.float32)       # [C,H]
    enc_w = np.zeros((D, EH), np.float16)
    dec_w = np.zeros((EH, D), np.float16)
    Wd = i["tail_dec_w"].astype(np.float32)       # [C,F,H]
    for c in range(C):
        enc_w[10 * c:10 * c + F, 8 * c:8 * c + H] = We[c].T  # [F,H]
        dec_w[8 * c:8 * c + H, 10 * c:10 * c + F] = Wd[c].T   # [H,F]
    red_w = np.zeros((D, 120 * 12), np.float16)
    for k in range(12):
        for c in range(C):
            red_w[10 * c:10 * c + F, 120 * k + 10 * k + c] = 0.1
    at = 1.0 / (i["head_nmax"].astype(np.float32) - i["head_nmin"] + EPS)  # [10]
    ct = -i["head_nmin"] * at
    vecs = np.zeros((128, 8), np.float32)
    vecs[0:D, 0] = aw.reshape(-1) * (1.0 / 255.0)   # dequant folded in
    vecs[0:D, 1] = cw.reshape(-1)
    vecs[0:EH, 2] = be.reshape(-1)
    vecs[0:D, 3] = i["tail_dec_b"].astype(np.float32).reshape(-1)
    vecs[0:120, 6] = np.tile(at, 12) * 255.0
    vecs[0:120, 7] = np.tile(ct, 12) * 255.0 + 0.5
    return dict(enc_w=enc_w, dec_w=dec_w, red_w=red_w, vecs=vecs)


def _get_runtime():
    if "rt" in _cached:
        return _cached["rt"]
    nc = _build_module()
    bass2jax.install_neuronx_cc_hook()
    partition_name = (nc.partition_id_tensor.name
                      if nc.partition_id_tensor is not None else None)
    in_names, out_names, out_avals = [], [], []
    for alloc in nc.m.functions[0].allocations:
        if not isinstance(alloc, mybir.MemoryLocationSet):
            continue
        name = alloc.memorylocations[0].name
        if alloc.kind == "ExternalInput":
            if name != partition_name:
                in_names.append(name)
        elif alloc.kind == "ExternalOutput":
            out_names.append(name)
            out_avals.append(jax.core.ShapedArray(
                tuple(alloc.tensor_shape), mybir.dt.np(alloc.dtype)))
    in_names_full = in_names + out_names + (
        [partition_name] if partition_name else [])

    def _body(*args):
        operands = list(args)
        if partition_name is not None:
            operands.append(bass2jax.partition_id_tensor())
        outs = bass2jax._bass_exec_p.bind(
            *operands, out_avals=tuple(out_avals),
            in_names=tuple(in_names_full), out_names=tuple(out_names),
            lowering_input_output_aliases=(),
            sim_require_finite=True, sim_require_nnan=True, nc=nc)
        return tuple(outs)

    devices = jax.devices()[:N_CORES]
    mesh = Mesh(np.asarray(devices), ("core",))
    nin = len(in_names) + len(out_names)
    fn = jax.jit(
        shard_map(_body, mesh=mesh, in_specs=(PartitionSpec("core"),) * nin,
                  out_specs=(PartitionSpec("core"),) * len(out_names),
                  check_rep=False),
        keep_unused=True)
    sh = NamedSharding(mesh, PartitionSpec("core"))
    # inert stand-in for the "pre-zeroed output" operand of bass_exec; the
    # kernel writes every output element so its contents are never read
    dummy = jax.jit(lambda: jnp.zeros((N_CORES * R2, 10), jnp.uint8),
                    out_shardings=sh)()
    dummy.block_until_ready()

    cpu = jax.devices("cpu")[0]

    # head MLP on XLA-CPU, straight from the uint8 device output (one call
    # per half-batch; weights are runtime args so this compiles once)
    def _head(o8, we, be_, wd, bd):
        t = o8.reshape(N_CORES, R2, 10).astype(jnp.float32) * jnp.float32(1 / 255)
        hh = jax.nn.sigmoid(t @ we.T + be_)
        return jax.nn.sigmoid(hh @ wd.T + bd)

    head = jax.jit(_head, backend="cpu")
    rt = dict(nc=nc, fn=fn, sh=sh, dummy=dummy, in_names=in_names, head=head,
              cpu=cpu, fetch=_cf.ThreadPoolExecutor(NH))
    _cached["rt"] = rt
    return rt


def _quant_half(x, h, xg):
    """Quantize+transpose rows [c*R + h*R2, c*R + (h+1)*R2) of each core into
    feature-major uint8 [800, R2]."""
    S = R2 // 2   # slice for cache-friendly fused quantize+transpose
    for c in range(N_CORES):
        base = c * R + h * R2
        for s in range(2):
            q = (x[base + s * S: base + (s + 1) * S] * np.float32(255)
                 + np.float32(0.5)).astype(np.uint8)
            xg[D * c:D * (c + 1), s * S:(s + 1) * S] = q.T
    return xg


def kernel(**inputs):
    try:
        return _kernel_once(inputs)
    except Exception:
        # transient device/transport failure: rebuild per-call device state
        # (fresh executors in case a task is wedged, re-upload params and the
        # dummy output operand) and retry once; the compiled jit is host-side
        # and survives
        rt = _cached.get("rt")
        if rt is not None:
            rt["fetch"] = _cf.ThreadPoolExecutor(NH)
            rt["dummy"] = jax.jit(
                lambda: jnp.zeros((N_CORES * R2, 10), jnp.uint8),
                out_shardings=rt["sh"])()
        _cached.pop("ph", None)
        _cached.pop("dev_params", None)
        # in-flight transfers from the failed attempt may still reference the
        # quantize buffers; retire them and start fresh
        _cached.pop("xg", None)
        return _kernel_once(inputs)


def _kernel_once(inputs):
    rt = _get_runtime()
    inputs = {k: np.asarray(v) for k, v in inputs.items()}

    # replicated parameters: upload once per distinct content
    hsh = hashlib.md5()
    for k in PARAM_KEYS:
        hsh.update(np.ascontiguousarray(inputs[k]).tobytes())
    ph = hsh.hexdigest()
    if _cached.get("ph") != ph:
        params = _fold_params(inputs)
        dev_params = {}
        for name in rt["in_names"]:
            if name == "x8":
                continue
            g = np.tile(params[name], (N_CORES, 1))
            dev_params[name] = jax.device_put(g, rt["sh"])
        for a in dev_params.values():
            a.block_until_ready()
        _cached["ph"] = ph
        _cached["dev_params"] = dev_params
    dev_params = _cached["dev_params"]

    # pipelined halves. device_put is async (returns in ~40ms while the
    # transfer streams), so the exec dispatch and the output-fetch await are
    # issued eagerly and pre-queue at the terminal behind the in-flight
    # transfer — no blocking anywhere on the host side. Buffer-reuse safety
    # comes from the data dependency: fetched outputs imply the exec consumed
    # its input, which implies the host->device read of xg completed.
    x = inputs["x"]
    if "xg" not in _cached:
        _cached["xg"] = [np.empty((N_CORES * D, R2), np.uint8)
                         for _ in range(NH)]
    fn, dummy, sh = rt["fn"], rt["dummy"], rt["sh"]
    arg_of = lambda xd: [xd if n == "x8" else dev_params[n]
                         for n in rt["in_names"]]

    fets = []
    for h in range(NH):
        _quant_half(x, h, _cached["xg"][h])   # overlaps earlier transfers
        (out,) = fn(*arg_of(jax.device_put(_cached["xg"][h], sh)), dummy)
        fets.append(rt["fetch"].submit(np.asarray, out))

    x_hat = np.empty((B, C), np.float32)
    t_out = np.empty((B, C), np.float32)
    hw = [inputs[k].astype(np.float32) for k in
          ("head_enc_w", "head_enc_b", "head_dec_w", "head_dec_b")]

    # chunk h's post-processing overlaps the later chunks' transfers
    for h in range(NH):
        o = fets[h].result()
        xh = rt["head"](o, *hw)   # async XLA-CPU dispatch
        for c in range(N_CORES):
            lo = c * R + h * R2
            np.multiply(o[c * R2:(c + 1) * R2], np.float32(1 / 255),
                        out=t_out[lo:lo + R2], casting="unsafe")
        xh = np.asarray(xh)
        for c in range(N_CORES):
            lo = c * R + h * R2
            x_hat[lo:lo + R2] = xh[c]
    return x_hat, t_out
